# revision 36
# baseline (speedup 1.0000x reference)
"""Trainium2 Bass kernel for nn_Encoder_46033459478804.

Pre-norm entmax15 multi-head self-attention + Mish FFN encoder layer.
B=8, S=1024, D=512, H=8, hd=64, F=2048, fp32 I/O.

Sharding: data-parallel over batch across 8 NeuronCores (1 batch each).
Each core runs the full encoder layer for its batch.

entmax15 is computed without sorting: the threshold tau solving
sum(relu(z - tau)^2) = 1 is found per row with a Gaussian-moment
initializer followed by Newton iterations (monotone from below for the
convex objective; quadratic convergence near the root). The first two
Newton updates estimate their sums from the first quarter / first half
of the keys respectively (rescaled; validated against the sort-based
reference), so only the final fused pass touches all S columns.

Layout strategy: activations are kept transposed ([d, t] with d on
partitions) so every matmul contraction lands on the partition axis,
with weights pre-transposed/pre-scaled on the host. Scores are produced
in [qt, kt] (query rows on partitions) so all entmax row reductions run
along the free axis (fused DVE scalar_tensor_tensor for relu+sum, ACT
Square / custom-DVE tensor_tensor_reduce for the squared sums). The
attention matrix is transposed head-by-head with the DMA xbar (bf16)
for the att @ v contraction. mish is composed as x*a/(a+2), a=e^x(e^x+2)
using ACT Exp + a fast custom-DVE reciprocal, spread over ACT/DVE/GPSIMD.
"""

import sys

for _p in ("/opt/trn_rl_repo", "/root/.axon_site/_ro/trn_rl_repo"):
    if _p not in sys.path:
        sys.path.insert(0, _p)

import numpy as np
import ml_dtypes
from contextlib import ExitStack

import concourse.bass as bass
import concourse.tile as tile
from concourse import bacc, mybir
from concourse.bass_utils import run_bass_kernel_spmd
from concourse.masks import make_identity
from concourse.dve_ops import TENSOR_TENSOR_REDUCE

F32 = mybir.dt.float32
BF16 = mybir.dt.bfloat16
AF = mybir.ActivationFunctionType
OP = mybir.AluOpType

P = 128
S = 1024
TO = S // P          # 8 token tiles
D = 512
DO = D // P          # 4 d tiles
H = 8
HD = D // H          # 64
F = 2048
FO = F // P          # 16 f tiles
EPS = 1e-5
NEWTON_ITERS = 3
FUSE_FINAL = True  # use last Newton iteration's r^2 as att (saves a full pass)
# u(w) cubic fit, w = sqrt(theta), theta = 1/(S * var);  tau0 = m - (u + .25)*sigma
UC3, UC2, UC1, UC0 = 9.46042885, -13.43179184, 8.58949291, -2.53359778 + 0.25
W_LO, W_HI = 0.0894, 0.5916


def _ln_T(nc, tc, src, dst, g_sb, b_sb, ones, ppsum_g, ppsum_1):
    """LayerNorm along d for transposed activations src [128, DO, S] fp32.

    Writes dst [128, DO, S]. Token mean/rstd come from PE ones-reductions
    over the partition axis; they are broadcast back across partitions
    with GPSIMD. Scratch pools are scoped to this call.
    """
    with tc.tile_pool(name="ln_scr", bufs=1) as pw, \
         tc.tile_pool(name="ln_small", bufs=1) as psm:
        sq = pw.tile([P, DO, S], F32, tag="ln_scr")
        nc.scalar.activation(out=sq[:], in_=src[:], func=AF.Square)

        bc_src = psm.tile([1, 2 * S], F32, tag="ln_bcsrc")
        mu = psm.tile([1, S], F32, tag="ln_mu")
        m2 = psm.tile([1, S], F32, tag="ln_m2")
        for ch in range(2):
            cs = slice(ch * 512, ch * 512 + 512)
            ps1 = ppsum_1.tile([1, 512], F32, tag="ln_ps")
            for do in range(DO):
                nc.tensor.matmul(out=ps1[:], lhsT=ones[:], rhs=src[:, do, cs],
                                 start=(do == 0), stop=(do == DO - 1))
            nc.vector.tensor_scalar(out=mu[:, cs], in0=ps1[:], scalar1=1.0 / D,
                                    scalar2=None, op0=OP.mult)
            ps2 = ppsum_1.tile([1, 512], F32, tag="ln_ps")
            for do in range(DO):
                nc.tensor.matmul(out=ps2[:], lhsT=ones[:], rhs=sq[:, do, cs],
                                 start=(do == 0), stop=(do == DO - 1))
            # m2 = s2/D + eps
            nc.vector.tensor_scalar(out=m2[:, cs], in0=ps2[:], scalar1=1.0 / D,
                                    scalar2=EPS, op0=OP.mult, op1=OP.add)
        musq = psm.tile([1, S], F32, tag="ln_musq")
        nc.vector.tensor_tensor(out=musq[:], in0=mu[:], in1=mu[:], op=OP.mult)
        var = psm.tile([1, S], F32, tag="ln_var")
        nc.vector.tensor_tensor(out=var[:], in0=m2[:], in1=musq[:], op=OP.subtract)
        sd = psm.tile([1, S], F32, tag="ln_sd")
        nc.scalar.activation(out=sd[:], in_=var[:], func=AF.Sqrt)
        # rho (+ one NR polish: ACT Sqrt can be low-precision)
        rho = bc_src[:, 0:S]
        nc.vector.reciprocal(out=rho, in_=sd[:])
        pol = psm.tile([1, S], F32, tag="ln_pol")
        nc.vector.tensor_tensor(out=pol[:], in0=rho, in1=rho, op=OP.mult)
        nc.vector.tensor_tensor(out=pol[:], in0=pol[:], in1=var[:], op=OP.mult)
        nc.vector.tensor_scalar(out=pol[:], in0=pol[:], scalar1=-0.5, scalar2=1.5,
                                op0=OP.mult, op1=OP.add)
        nc.vector.tensor_tensor(out=rho, in0=rho, in1=pol[:], op=OP.mult)
        nc.vector.tensor_tensor(out=bc_src[:, S:2 * S], in0=mu[:], in1=rho,
                                op=OP.mult)
        bc = pw.tile([P, 2 * S], F32, tag="ln_bc")
        nc.gpsimd.partition_broadcast(bc[:], bc_src[:])

        tmp = pw.tile([P, DO, S], F32, tag="ln_scr")
        for do in range(DO):
            # normalize: split across GPSIMD (muls) and DVE (affine)
            nc.gpsimd.tensor_tensor(out=tmp[:, do, :], in0=src[:, do, :],
                                    in1=bc[:, 0:S], op=OP.mult)
            nc.gpsimd.tensor_tensor(out=tmp[:, do, :], in0=tmp[:, do, :],
                                    in1=bc[:, S:2 * S], op=OP.subtract)
            nc.vector.tensor_scalar(out=dst[:, do, :], in0=tmp[:, do, :],
                                    scalar1=g_sb[:, do:do + 1],
                                    scalar2=b_sb[:, do:do + 1],
                                    op0=OP.mult, op1=OP.add)


_DEBUG = False

_WPACK_BF = [("wqT", (P, DO, D)), ("wkT", (P, DO, D)), ("wvT", (P, DO, D)),
             ("woT", (P, DO, D)), ("w1T", (P, DO, F)), ("w2T", (P, FO, D))]
_WPACK_F32 = [("bq", (P, DO)), ("bk", (P, DO)), ("bv", (1, D)),
              ("bo", (P, DO)), ("bf1", (P, FO)), ("bf2", (P, DO)),
              ("g1", (P, DO)), ("b1", (P, DO)), ("g2", (P, DO)),
              ("b2", (P, DO)), ("gf", (P, DO)), ("bf", (P, DO))]


def _emit(nc, n_iters=1):
    """Emit the full encoder program. n_iters > 1 repeats the ENTIRE body
    (including every weight DMA) back-to-back on the same DRAM in/out
    buffers; used only by the timing harness to measure per-iteration HW
    time with the fixed per-execute dispatch overhead amortized away."""
    taps = {}

    def tap(name, ap_or_tile, shape, dt):
        if not _DEBUG:
            return
        d = nc.dram_tensor("tap_" + name, shape, dt, kind="ExternalOutput").ap()
        nc.sync.dma_start(d, ap_or_tile)
        taps[name] = d

    x_d = nc.dram_tensor("x", [S, D], F32, kind="ExternalInput").ap()
    # all weights packed into two flat buffers — the axon tunnel charges
    # ~90 ms per buffer round-trip, so fewer buffers >> anything else
    nbf = sum(int(np.prod(s)) for _, s in _WPACK_BF)
    nf = sum(int(np.prod(s)) for _, s in _WPACK_F32)
    wbf_d = nc.dram_tensor("wbf", [nbf], BF16, kind="ExternalInput").ap()
    wf_d = nc.dram_tensor("wf", [nf], F32, kind="ExternalInput").ap()

    def _slices(flat, pack):
        out, off = {}, 0
        for name, shape in pack:
            n = int(np.prod(shape))
            sl = flat[off:off + n]
            if len(shape) == 3:
                sl = sl.rearrange("(p o m) -> p o m", p=shape[0], o=shape[1])
            else:
                sl = sl.rearrange("(p o) -> p o", p=shape[0])
            out[name] = sl
            off += n
        return out

    dbf = _slices(wbf_d, _WPACK_BF)
    df = _slices(wf_d, _WPACK_F32)
    (wqT_d, wkT_d, wvT_d, woT_d, w1T_d, w2T_d) = (
        dbf["wqT"], dbf["wkT"], dbf["wvT"], dbf["woT"], dbf["w1T"], dbf["w2T"])
    (bq_d, bk_d, bv_d, bo_d, bf1_d, bf2_d, g1_d, b1_d, g2_d, b2_d, gf_d,
     bf_d) = (df["bq"], df["bk"], df["bv"], df["bo"], df["bf1"], df["bf2"],
              df["g1"], df["b1"], df["g2"], df["b2"], df["gf"], df["bf"])
    out_d = nc.dram_tensor("out", [S, D], F32, kind="ExternalOutput").ap()

    def _one_iter(tc):
      with ExitStack() as ctx:
        ppersist = ctx.enter_context(tc.tile_pool(name="persist", bufs=1))
        psmall = ctx.enter_context(tc.tile_pool(name="small", bufs=2))
        ppsum_z = ctx.enter_context(tc.tile_pool(name="psz", bufs=2, space="PSUM"))
        ppsum_g = ctx.enter_context(tc.tile_pool(name="psg", bufs=2, space="PSUM"))
        ppsum_1 = ctx.enter_context(tc.tile_pool(name="ps1", bufs=2, space="PSUM"))

        def load(pool, dram, shape, dt, tag):
            t = pool.tile(shape, dt, tag=tag)
            nc.sync.dma_start(t[:], dram)
            return t

        # ---- long-lived constants / tensors ----
        wo = load(ppersist, woT_d, [P, DO, D], BF16, "wo")
        bq = load(ppersist, bq_d, [P, DO], F32, "bq")
        bk = load(ppersist, bk_d, [P, DO], F32, "bk")
        bo = load(ppersist, bo_d, [P, DO], F32, "bo")
        bf1 = load(ppersist, bf1_d, [P, FO], F32, "bf1")
        bf2 = load(ppersist, bf2_d, [P, DO], F32, "bf2")
        g1 = load(ppersist, g1_d, [P, DO], F32, "g1")
        b1 = load(ppersist, b1_d, [P, DO], F32, "b1")
        g2 = load(ppersist, g2_d, [P, DO], F32, "g2")
        b2 = load(ppersist, b2_d, [P, DO], F32, "b2")
        gf = load(ppersist, gf_d, [P, DO], F32, "gf")
        bf = load(ppersist, bf_d, [P, DO], F32, "bf")
        bv1 = load(ppersist, bv_d, [1, D], F32, "bv1")
        bvbc = ppersist.tile([P, D], F32, tag="bvbc")
        nc.gpsimd.partition_broadcast(bvbc[:], bv1[:])

        ident = ppersist.tile([P, P], F32, tag="ident")
        make_identity(nc, ident[:])
        ones = ppersist.tile([P, 1], F32, tag="ones")
        nc.vector.memset(ones[:], 1.0)
        zer_bf = ppersist.tile([P, S], BF16, tag="zer_bf")
        nc.vector.memset(zer_bf[:], 0.0)

        xT = ppersist.tile([P, DO, S], F32, tag="xT")          # reused as x3T
        x2T = ppersist.tile([P, DO, S], F32, tag="x2T")        # reused as outT
        attoutT = ppersist.tile([P, DO, S], BF16, tag="attoutT")

        with tc.tile_pool(name="attn_live", bufs=1) as pal:
            # ---- x load + transpose ----
            x_nat = pal.tile([P, TO, D], F32, tag="x_nat")
            nc.sync.dma_start(x_nat[:], x_d.rearrange("(to p) d -> p to d", p=P))
            for to in range(TO):
                for do in range(DO):
                    pt = ppsum_g.tile([P, 512], F32, tag="pg")
                    nc.tensor.transpose(pt[:, :P], x_nat[:, to, do * P:(do + 1) * P],
                                        ident[:])
                    nc.vector.tensor_copy(xT[:, do, to * P:(to + 1) * P], pt[:, :P])

            # ---- LN1 -> y (bf16, transposed) ----
            y_bf = pal.tile([P, DO, S], BF16, tag="y_bf")
            _ln_T(nc, tc, xT, y_bf, g1, b1, ones, ppsum_g, ppsum_1)
            tap("xT", xT[:], [P, DO, S], F32)
            tap("y_bf", y_bf[:], [P, DO, S], BF16)

            # ---- projections ----
            wq = load(pal, wqT_d, [P, DO, D], BF16, "wq")
            wk = load(pal, wkT_d, [P, DO, D], BF16, "wk")
            wv = load(pal, wvT_d, [P, DO, D], BF16, "wv")
            qT = pal.tile([P, DO, S], BF16, tag="qT")
            kT = pal.tile([P, DO, S], BF16, tag="kT")
            for (wmat, bias, dst) in ((wq, bq, qT), (wk, bk, kT)):
                for dt in range(DO):
                    for ch in range(2):
                        cs = slice(ch * 512, ch * 512 + 512)
                        ps = ppsum_g.tile([P, 512], F32, tag="pg")
                        for di in range(DO):
                            nc.tensor.matmul(
                                out=ps[:], lhsT=wmat[:, di, dt * P:(dt + 1) * P],
                                rhs=y_bf[:, di, cs],
                                start=(di == 0), stop=(di == DO - 1))
                        nc.vector.tensor_scalar(out=dst[:, dt, cs], in0=ps[:],
                                                scalar1=bias[:, dt:dt + 1],
                                                scalar2=None, op0=OP.add)
            v_bf = pal.tile([P, TO, D], BF16, tag="v_bf")
            for tt in range(TO):
                ps = ppsum_g.tile([P, 512], F32, tag="pg")
                for di in range(DO):
                    nc.tensor.matmul(out=ps[:], lhsT=y_bf[:, di, tt * P:(tt + 1) * P],
                                     rhs=wv[:, di, :],
                                     start=(di == 0), stop=(di == DO - 1))
                nc.vector.tensor_tensor(out=v_bf[:, tt, :], in0=ps[:], in1=bvbc[:],
                                        op=OP.add)
            tap("qT", qT[:], [P, DO, S], BF16)
            tap("kT", kT[:], [P, DO, S], BF16)
            tap("v_bf", v_bf[:], [P, TO, D], BF16)

            # ---- attention ----
            with tc.tile_pool(name="z", bufs=2) as pz, \
                 tc.tile_pool(name="attT", bufs=1) as pattT, \
                 tc.tile_pool(name="r", bufs=3) as pr, \
                 tc.tile_pool(name="r2", bufs=2) as pr2, \
                 tc.tile_pool(name="att", bufs=3) as patt:

                def emit_scores(h):
                    """q@kT for head h -> z bf16 [128, TO, S]; rows qt, cols kt."""
                    bp = (h % 2) * HD
                    doh = h // 2
                    q_l = qT[bp:bp + HD, doh, :]
                    k_r = kT[bp:bp + HD, doh, :]
                    z = pz.tile([P, TO, S], BF16, tag="z")
                    s1z = psmall.tile([P, TO], F32, tag="s1z")
                    s2z = psmall.tile([P, TO], F32, tag="s2z")
                    for qt in range(TO):
                        ps = ppsum_z.tile([P, S], F32, tag="pz")
                        for kc in range(2):
                            cs = slice(kc * 512, kc * 512 + 512)
                            nc.tensor.matmul(out=ps[:, cs],
                                             lhsT=q_l[:, qt * P:(qt + 1) * P],
                                             rhs=k_r[:, cs], start=True, stop=True)
                        if qt % 2 == 0:
                            nc.scalar.activation(out=z[:, qt, :], in_=ps[:],
                                                 func=AF.Copy,
                                                 accum_out=s1z[:, qt:qt + 1])
                        else:
                            nc.vector.tensor_scalar(out=z[:, qt, :], in0=ps[:],
                                                    scalar1=1.0, scalar2=0.0,
                                                    op0=OP.mult, op1=OP.add,
                                                    accum_out=s1z[:, qt:qt + 1])
                        sub = pr.tile([P, S // 8], BF16, tag="sub")
                        nc.vector._custom_dve(
                            TENSOR_TENSOR_REDUCE, out=sub[:], in0=z[:, qt, ::8],
                            in1=z[:, qt, ::8], s0=0.0, s1=1.0,
                            accum_out=s2z[:, qt:qt + 1])
                    return z, s1z, s2z

                def emit_entmax(h, z, s1z, s2z):
                    # --- init: tau0 = m - u(w)*sigma,  w = sqrt(1/(S*var)) ---
                    m = psmall.tile([P, TO], F32, tag="tm")
                    nc.vector.tensor_scalar(out=m[:], in0=s1z[:], scalar1=1.0 / S,
                                            scalar2=None, op0=OP.mult)
                    msq = psmall.tile([P, TO], F32, tag="tmsq")
                    nc.vector.tensor_tensor(out=msq[:], in0=m[:], in1=m[:], op=OP.mult)
                    var = psmall.tile([P, TO], F32, tag="tvar")
                    nc.vector.tensor_scalar(out=var[:], in0=s2z[:], scalar1=8.0 / S,
                                            scalar2=None, op0=OP.mult)
                    nc.vector.tensor_tensor(out=var[:], in0=var[:], in1=msq[:],
                                            op=OP.subtract)
                    nc.vector.tensor_scalar(out=var[:], in0=var[:], scalar1=1e-8,
                                            scalar2=None, op0=OP.max)
                    th = psmall.tile([P, TO], F32, tag="tth")
                    nc.vector.reciprocal(out=th[:], in_=var[:])
                    nc.vector.tensor_scalar(out=th[:], in0=th[:], scalar1=1.0 / S,
                                            scalar2=None, op0=OP.mult)
                    w = psmall.tile([P, TO], F32, tag="tw")
                    nc.scalar.activation(out=w[:], in_=th[:], func=AF.Sqrt)
                    nc.vector.tensor_scalar(out=w[:], in0=w[:], scalar1=W_LO,
                                            scalar2=W_HI, op0=OP.max, op1=OP.min)
                    sg = psmall.tile([P, TO], F32, tag="tsg")
                    nc.scalar.activation(out=sg[:], in_=var[:], func=AF.Sqrt)
                    u = psmall.tile([P, TO], F32, tag="tu")
                    nc.vector.tensor_scalar(out=u[:], in0=w[:], scalar1=UC3,
                                            scalar2=UC2, op0=OP.mult, op1=OP.add)
                    nc.vector.tensor_tensor(out=u[:], in0=u[:], in1=w[:], op=OP.mult)
                    nc.vector.tensor_scalar(out=u[:], in0=u[:], scalar1=UC1,
                                            scalar2=None, op0=OP.add)
                    nc.vector.tensor_tensor(out=u[:], in0=u[:], in1=w[:], op=OP.mult)
                    nc.vector.tensor_scalar(out=u[:], in0=u[:], scalar1=UC0,
                                            scalar2=None, op0=OP.add)
                    # keep tau NEGATED: tau_neg = u*sigma - m, so both DVE
                    # (STT op0=add) and ACT (bias=tau_neg) can consume it
                    tau = psmall.tile([P, TO], F32, tag="tau")
                    nc.vector.tensor_tensor(out=tau[:], in0=u[:], in1=sg[:],
                                            op=OP.mult)
                    nc.vector.tensor_tensor(out=tau[:], in0=tau[:], in1=m[:],
                                            op=OP.subtract)

                    # --- Newton iterations ---
                    # Engine split per tile: qt<4 -> ACT Relu(+S1) then DVE
                    # custom-TTR square(+S2); qt>=4 -> DVE STT relu(+S1) then
                    # ACT Square(+S2). 8 big ops per engine per head-iter.
                    attT = pattT.tile([P, TO, S], BF16, tag="attT")
                    for it in range(NEWTON_ITERS):
                        last = FUSE_FINAL and (it == NEWTON_ITERS - 1)
                        # progressive sampling: quarter, half, full keys --
                        # early Newton steps tolerate sampled sums (validated
                        # vs sort-based entmax: rms unchanged)
                        ncols = S if last else (S // 4 if it == 0 else S // 2)
                        s1 = psmall.tile([P, TO], F32, tag="ns1")
                        s2 = psmall.tile([P, TO], F32, tag="ns2")
                        for qt in range(TO):
                            r = pr.tile([P, S], BF16, tag="r")
                            rv = r[:, 0:ncols]
                            zv = z[:, qt, 0:ncols]
                            if qt < 4:
                                # r = relu(z + tau_neg) on ACT, S1 accumulated
                                nc.scalar.activation(
                                    out=rv, in_=zv, func=AF.Relu,
                                    bias=tau[:, qt:qt + 1], scale=1.0,
                                    accum_out=s1[:, qt:qt + 1])
                            else:
                                nc.vector.scalar_tensor_tensor(
                                    out=rv, in0=zv,
                                    scalar=tau[:, qt:qt + 1],
                                    in1=zer_bf[:, 0:ncols],
                                    op0=OP.add, op1=OP.max,
                                    accum_out=s1[:, qt:qt + 1])
                            if last:
                                r2 = patt.tile([P, S], BF16, tag="arow")
                            elif qt < 4:
                                r2 = patt.tile([P, S], BF16, tag="arow")
                            else:
                                r2 = pr2.tile([P, S], F32, tag="r2f")
                            r2v = r2[:, 0:ncols]
                            if qt < 4:
                                nc.vector._custom_dve(
                                    TENSOR_TENSOR_REDUCE, out=r2v, in0=rv,
                                    in1=rv, s0=0.0, s1=1.0,
                                    accum_out=s2[:, qt:qt + 1])
                            else:
                                nc.scalar.activation(out=r2v, in_=rv,
                                                     func=AF.Square,
                                                     accum_out=s2[:, qt:qt + 1])
                            if last:
                                nc.sync.dma_start_transpose(
                                    attT[:, :, qt * P:(qt + 1) * P], r2[:])
                        if last:
                            break
                        # tau_neg -= clip((s2-c)/(2*s1), 0, 0.25); the it==0
                        # pass sums only the first half of the keys, so its
                        # sums estimate half the full values: c = 0.5.
                        cnum = -0.125 if it == 0 else -0.25
                        rcp = psmall.tile([P, TO], F32, tag="nrcp")
                        nc.vector.reciprocal(out=rcp[:], in_=s1[:])
                        num = psmall.tile([P, TO], F32, tag="nnum")
                        nc.vector.tensor_scalar(out=num[:], in0=s2[:], scalar1=0.5,
                                                scalar2=cnum, op0=OP.mult, op1=OP.add)
                        step = psmall.tile([P, TO], F32, tag="nstep")
                        nc.vector.tensor_tensor(out=step[:], in0=num[:], in1=rcp[:],
                                                op=OP.mult)
                        nc.vector.tensor_scalar(out=step[:], in0=step[:], scalar1=0.0,
                                                scalar2=0.25, op0=OP.max, op1=OP.min)
                        nc.vector.tensor_tensor(out=tau[:], in0=tau[:], in1=step[:],
                                                op=OP.subtract)
                    return attT

                def emit_attv(h, attT):
                    bp = (h % 2) * HD
                    doh = h // 2
                    for ch in range(2):
                        cs = slice(ch * 512, ch * 512 + 512)
                        ps = ppsum_g.tile([P, 512], F32, tag="pg")
                        for kto in range(TO):
                            nc.tensor.matmul(out=ps[:HD, :],
                                             lhsT=v_bf[:, kto, h * HD:(h + 1) * HD],
                                             rhs=attT[:, kto, cs],
                                             start=(kto == 0), stop=(kto == TO - 1))
                        nc.vector.tensor_copy(attoutT[bp:bp + HD, doh, cs], ps[:HD, :])

                pending = (0,) + emit_scores(0)
                for h in range(H):
                    _, z, s1z, s2z = pending
                    if h == 0:
                        tap("z0", z[:], [P, TO, S], BF16)
                        tap("s1z0", s1z[:], [P, TO], F32)
                        tap("s2z0", s2z[:], [P, TO], F32)
                    attT = emit_entmax(h, z, s1z, s2z)
                    if h == 0:
                        tap("attT0", attT[:], [P, TO, S], BF16)
                    if h + 1 < H:
                        pending = (h + 1,) + emit_scores(h + 1)
                    emit_attv(h, attT)
                tap("attoutT", attoutT[:], [P, DO, S], BF16)

        # ---- output projection + residual: x2T = xT + woT.T @ attoutT + bo ----
        for dt in range(DO):
            for ch in range(2):
                cs = slice(ch * 512, ch * 512 + 512)
                ps = ppsum_g.tile([P, 512], F32, tag="pg")
                for di in range(DO):
                    nc.tensor.matmul(out=ps[:], lhsT=wo[:, di, dt * P:(dt + 1) * P],
                                     rhs=attoutT[:, di, cs],
                                     start=(di == 0), stop=(di == DO - 1))
                nc.vector.scalar_tensor_tensor(
                    out=x2T[:, dt, cs], in0=ps[:], scalar=bo[:, dt:dt + 1],
                    in1=xT[:, dt, cs], op0=OP.add, op1=OP.add)

        with tc.tile_pool(name="ffn_live", bufs=1) as pfl:
            w1 = load(pfl, w1T_d, [P, DO, F], BF16, "w1")
            w2 = load(pfl, w2T_d, [P, FO, D], BF16, "w2")
            y2_bf = pfl.tile([P, DO, S], BF16, tag="y2_bf")
            hT = pfl.tile([P, FO, S], BF16, tag="hT")

            tap("x2T", x2T[:], [P, DO, S], F32)
            # ---- LN2 -> y2 ----
            _ln_T(nc, tc, x2T, y2_bf, g2, b2, ones, ppsum_g, ppsum_1)

            # ---- FFN in: hT = mish(w1T.T @ y2 + bf1) ----
            with tc.tile_pool(name="mish", bufs=2) as pm:
                for fo in range(FO):
                    for ch in range(2):
                        cs = slice(ch * 512, ch * 512 + 512)
                        ps = ppsum_g.tile([P, 512], F32, tag="pg")
                        for di in range(DO):
                            nc.tensor.matmul(
                                out=ps[:], lhsT=w1[:, di, fo * P:(fo + 1) * P],
                                rhs=y2_bf[:, di, cs],
                                start=(di == 0), stop=(di == DO - 1))
                        e = pm.tile([P, 512], F32, tag="m_e")
                        nc.scalar.activation(out=e[:], in_=ps[:], func=AF.Exp,
                                             bias=bf1[:, fo:fo + 1], scale=1.0)
                        xb = pm.tile([P, 512], F32, tag="m_xb")
                        nc.scalar.activation(out=xb[:], in_=ps[:], func=AF.Identity,
                                             bias=bf1[:, fo:fo + 1], scale=1.0)
                        a = pm.tile([P, 512], F32, tag="m_a")
                        nc.vector.scalar_tensor_tensor(out=a[:], in0=e[:], scalar=2.0,
                                                       in1=e[:], op0=OP.add,
                                                       op1=OP.mult)
                        d = pm.tile([P, 512], F32, tag="m_d")
                        nc.vector.tensor_scalar(out=d[:], in0=a[:], scalar1=2.0,
                                                scalar2=None, op0=OP.add)
                        rc = pm.tile([P, 512], F32, tag="m_rc")
                        nc.vector.reciprocal_approx_fast(out=rc[:], in_=d[:])
                        p1 = pm.tile([P, 512], F32, tag="m_p1")
                        nc.gpsimd.tensor_tensor(out=p1[:], in0=xb[:], in1=a[:],
                                                op=OP.mult)
                        nc.gpsimd.tensor_tensor(out=hT[:, fo, cs], in0=p1[:],
                                                in1=rc[:], op=OP.mult)

            # ---- FFN out + residual: x3T = x2T + w2T.T @ hT + bf2 ----
            x3T = ppersist.tile([P, DO, S], F32, tag="xT")  # reuse xT slot
            for dt in range(DO):
                for ch in range(2):
                    cs = slice(ch * 512, ch * 512 + 512)
                    ps = ppsum_g.tile([P, 512], F32, tag="pg")
                    for fo in range(FO):
                        nc.tensor.matmul(out=ps[:],
                                         lhsT=w2[:, fo, dt * P:(dt + 1) * P],
                                         rhs=hT[:, fo, cs],
                                         start=(fo == 0), stop=(fo == FO - 1))
                    nc.vector.scalar_tensor_tensor(
                        out=x3T[:, dt, cs], in0=ps[:], scalar=bf2[:, dt:dt + 1],
                        in1=x2T[:, dt, cs], op0=OP.add, op1=OP.add)

            tap("hT", hT[:], [P, FO, S], BF16)
            tap("x3T", x3T[:], [P, DO, S], F32)

            # ---- final LN (outT reuses the x2T slot) ----
            outT = ppersist.tile([P, DO, S], F32, tag="x2T")
            _ln_T(nc, tc, x3T, outT, gf, bf, ones, ppsum_g, ppsum_1)

            # ---- transpose back + store ----
            with tc.tile_pool(name="outp", bufs=1) as po:
                out_nat = po.tile([P, TO, D], F32, tag="out_nat")
                for to in range(TO):
                    for do in range(DO):
                        pt = ppsum_g.tile([P, 512], F32, tag="pg")
                        nc.tensor.transpose(pt[:, :P],
                                            outT[:, do, to * P:(to + 1) * P],
                                            ident[:])
                        nc.vector.tensor_copy(out_nat[:, to, do * P:(do + 1) * P],
                                              pt[:, :P])
                nc.sync.dma_start(out_d.rearrange("(to p) d -> p to d", p=P),
                                  out_nat[:])

    with tile.TileContext(nc) as tc:
        for _ in range(n_iters):
            _one_iter(tc)

    return nc


_CACHE = {}


def _get_nc(n_iters=1):
    key = "nc" if n_iters == 1 else f"nc{n_iters}"
    if key not in _CACHE:
        nc = bacc.Bacc("TRN2", target_bir_lowering=False, debug=False)
        _emit(nc, n_iters=n_iters)
        nc.compile()
        _CACHE[key] = nc
    return _CACHE[key]


def _prep_weights(inputs):
    bf = ml_dtypes.bfloat16
    c = 1.0 / 16.0  # 1/(2*sqrt(hd)) folded into q

    def tr(w):  # [dout, din] -> [din(P,O), dout]
        wt = np.ascontiguousarray(np.asarray(w, dtype=np.float32).T)
        o = wt.shape[0] // P
        return np.ascontiguousarray(wt.reshape(o, P, -1).transpose(1, 0, 2))

    def col(v):  # [n] -> [P, n//P] per-partition layout
        return np.ascontiguousarray(
            np.asarray(v, dtype=np.float32).reshape(-1, P).T)

    vals = {
        "wqT": tr(np.asarray(inputs["Wq"]) * c).astype(bf),
        "wkT": tr(inputs["Wk"]).astype(bf),
        "wvT": tr(inputs["Wv"]).astype(bf),
        "woT": tr(inputs["Wo"]).astype(bf),
        "w1T": tr(inputs["W1"]).astype(bf),
        "w2T": tr(inputs["W2"]).astype(bf),
        "bq": col(np.asarray(inputs["bq"]) * c),
        "bk": col(inputs["bk"]),
        "bv": np.asarray(inputs["bv"], dtype=np.float32).reshape(1, -1).copy(),
        "bo": col(inputs["bo"]),
        "bf1": col(inputs["bf1"]),
        "bf2": col(inputs["bf2"]),
        "g1": col(inputs["ln1_g"]),
        "b1": col(inputs["ln1_b"]),
        "g2": col(inputs["ln2_g"]),
        "b2": col(inputs["ln2_b"]),
        "gf": col(inputs["lnf_g"]),
        "bf": col(inputs["lnf_b"]),
    }
    wbf = np.concatenate([np.ascontiguousarray(vals[n]).ravel()
                          for n, _ in _WPACK_BF])
    wf = np.concatenate([np.ascontiguousarray(vals[n]).ravel()
                         for n, _ in _WPACK_F32])
    return {"wbf": wbf, "wf": wf}


def _get_runner(n_cores, n_iters=1):
    """Build the shard_map'd jit callable once and reuse it across calls
    (run_bass_via_pjrt re-traces per call, which costs ~100ms)."""
    key = ("runner", n_cores, n_iters)
    if key in _CACHE:
        return _CACHE[key]
    import jax
    import numpy as _np
    from jax.sharding import Mesh, PartitionSpec
    from jax.experimental.shard_map import shard_map
    from concourse import bass2jax as b2j
    from concourse import mybir as mb

    nc = _get_nc(n_iters)
    b2j.install_neuronx_cc_hook()
    pid_name = nc.partition_id_tensor.name if nc.partition_id_tensor else None
    in_names, out_names, out_avals, zero_shapes = [], [], [], []
    for alloc in nc.m.functions[0].allocations:
        if not isinstance(alloc, mb.MemoryLocationSet):
            continue
        name = alloc.memorylocations[0].name
        if alloc.kind == "ExternalInput":
            if name != pid_name:
                in_names.append(name)
        elif alloc.kind == "ExternalOutput":
            out_names.append(name)
            shape = tuple(alloc.tensor_shape)
            dtype = mb.dt.np(alloc.dtype)
            out_avals.append(jax.core.ShapedArray(shape, dtype))
            zero_shapes.append((shape, dtype))
    n_params = len(in_names)
    all_names = in_names + out_names
    if pid_name is not None:
        all_names = all_names + [pid_name]
    donate = tuple(range(n_params, n_params + len(out_names)))

    def _body(*args):
        operands = list(args)
        if pid_name is not None:
            operands.append(b2j.partition_id_tensor())
        outs = b2j._bass_exec_p.bind(
            *operands,
            out_avals=tuple(out_avals),
            in_names=tuple(all_names),
            out_names=tuple(out_names),
            lowering_input_output_aliases=(),
            sim_require_finite=True,
            sim_require_nnan=True,
            nc=nc,
        )
        return tuple(outs)

    devices = jax.devices()[:n_cores]
    mesh = Mesh(_np.asarray(devices), ("core",))
    # only "x" differs per core; every weight/bias is replicated so the
    # host->device upload ships one copy instead of n_cores concatenated ones
    sharded_names = {"x"}
    in_specs = tuple(
        PartitionSpec("core") if n in sharded_names else PartitionSpec()
        for n in in_names
    ) + (PartitionSpec("core"),) * len(out_names)
    sharded = jax.jit(
        shard_map(_body, mesh=mesh, in_specs=in_specs,
                  out_specs=(PartitionSpec("core"),) * len(out_names),
                  check_rep=False),
        donate_argnums=donate, keep_unused=True)

    # donated output buffers are created ON DEVICE (the kernel writes every
    # output element, so their content is irrelevant; uploading 16 MB of
    # host zeros per call would cost ~200 ms through the axon tunnel)
    from jax.sharding import NamedSharding
    import jax.numpy as jnp
    zshard = NamedSharding(mesh, PartitionSpec("core"))
    zeros_maker = jax.jit(
        lambda: tuple(jnp.zeros((n_cores * s[0],) + tuple(s[1:]), dt)
                      for (s, dt) in zero_shapes),
        out_shardings=(zshard,) * len(zero_shapes))

    runner = (sharded, in_names, out_names, zero_shapes, n_cores, sharded_names,
              zeros_maker)
    _CACHE[key] = runner
    return runner


def _run(in_maps):
    import numpy as _np
    (sharded, in_names, out_names, zero_shapes, n_cores, sharded_names,
     zeros_maker) = _get_runner(len(in_maps))
    concat_in = [
        _np.concatenate([_np.asarray(m[name]) for m in in_maps], axis=0)
        if name in sharded_names else _np.asarray(in_maps[0][name])
        for name in in_names
    ]
    zeros = zeros_maker()
    outs = sharded(*concat_in, *zeros)
    res = []
    for c in range(n_cores):
        d = {}
        for i, name in enumerate(out_names):
            arr = _np.asarray(outs[i])
            per = arr.shape[0] // n_cores
            d[name] = arr[c * per:(c + 1) * per]
        res.append(d)
    return res


def kernel(**inputs) -> np.ndarray:
    x = np.asarray(inputs["x"], dtype=np.float32)
    B = x.shape[0]
    shared = _prep_weights(inputs)
    in_maps = []
    for b in range(B):
        m = dict(shared)
        m["x"] = np.ascontiguousarray(x[b])
        in_maps.append(m)
    results = _run(in_maps)
    out = np.stack([results[b]["out"] for b in range(B)], axis=0)
    return out.astype(np.float32)


if __name__ == "__main__":
    import reference
    inputs = reference.setup_inputs()
    outs = kernel(**{k: np.asarray(v) for k, v in inputs.items()})
    print("kernel output:", outs.shape, outs.dtype)



# revision 37
# speedup vs baseline: 1.7983x; 1.7983x over previous
"""Trainium2 Bass kernel for nn_Encoder_46033459478804.

Pre-norm entmax15 multi-head self-attention + Mish FFN encoder layer.
B=8, S=1024, D=512, H=8, hd=64, F=2048, fp32 I/O.

Sharding: data-parallel over batch across 8 NeuronCores (1 batch each).
Each core runs the full encoder layer for its batch.

entmax15 is computed without sorting: the threshold tau solving
sum(relu(z - tau)^2) = 1 is found per row with a Gaussian-moment
initializer followed by Newton iterations (monotone from below for the
convex objective; quadratic convergence near the root). The first two
Newton updates estimate their sums from the first quarter / first half
of the keys respectively (rescaled; validated against the sort-based
reference), so only the final fused pass touches all S columns.

Layout strategy: activations are kept transposed ([d, t] with d on
partitions) so every matmul contraction lands on the partition axis,
with weights pre-transposed/pre-scaled on the host. Scores are produced
in [qt, kt] (query rows on partitions) so all entmax row reductions run
along the free axis (fused DVE scalar_tensor_tensor for relu+sum, ACT
Square / custom-DVE tensor_tensor_reduce for the squared sums). The
attention matrix is transposed head-by-head with the DMA xbar (bf16)
for the att @ v contraction. mish is composed as x*a/(a+2), a=e^x(e^x+2)
using ACT Exp + a fast custom-DVE reciprocal, spread over ACT/DVE/GPSIMD.
"""

import sys

for _p in ("/opt/trn_rl_repo", "/root/.axon_site/_ro/trn_rl_repo"):
    if _p not in sys.path:
        sys.path.insert(0, _p)

import numpy as np
import ml_dtypes
from contextlib import ExitStack

import concourse.bass as bass
import concourse.tile as tile
from concourse import bacc, mybir
from concourse.bass_utils import run_bass_kernel_spmd
from concourse.masks import make_identity
from concourse.dve_ops import TENSOR_TENSOR_REDUCE

F32 = mybir.dt.float32
BF16 = mybir.dt.bfloat16
AF = mybir.ActivationFunctionType
OP = mybir.AluOpType

P = 128
S = 1024
TO = S // P          # 8 token tiles
D = 512
DO = D // P          # 4 d tiles
H = 8
HD = D // H          # 64
F = 2048
FO = F // P          # 16 f tiles
EPS = 1e-5
NEWTON_ITERS = 3
FUSE_FINAL = True  # use last Newton iteration's r^2 as att (saves a full pass)
# u(w) cubic fit, w = sqrt(theta), theta = 1/(S * var);  tau0 = m - (u + .25)*sigma
UC3, UC2, UC1, UC0 = 9.46042885, -13.43179184, 8.58949291, -2.53359778 + 0.25
W_LO, W_HI = 0.0894, 0.5916


def _ln_T(nc, tc, src, dst, g_sb, b_sb, ones, ppsum_g, ppsum_1):
    """LayerNorm along d for transposed activations src [128, DO, S] fp32.

    Writes dst [128, DO, S]. Token mean/rstd come from PE ones-reductions
    over the partition axis; they are broadcast back across partitions
    with GPSIMD. Scratch pools are scoped to this call.
    """
    with tc.tile_pool(name="ln_scr", bufs=1) as pw, \
         tc.tile_pool(name="ln_small", bufs=1) as psm:
        sq = pw.tile([P, DO, S], F32, tag="ln_scr")
        nc.scalar.activation(out=sq[:], in_=src[:], func=AF.Square)

        bc_src = psm.tile([1, 2 * S], F32, tag="ln_bcsrc")
        mu = psm.tile([1, S], F32, tag="ln_mu")
        m2 = psm.tile([1, S], F32, tag="ln_m2")
        for ch in range(2):
            cs = slice(ch * 512, ch * 512 + 512)
            ps1 = ppsum_1.tile([1, 512], F32, tag="ln_ps")
            for do in range(DO):
                nc.tensor.matmul(out=ps1[:], lhsT=ones[:], rhs=src[:, do, cs],
                                 start=(do == 0), stop=(do == DO - 1))
            nc.vector.tensor_scalar(out=mu[:, cs], in0=ps1[:], scalar1=1.0 / D,
                                    scalar2=None, op0=OP.mult)
            ps2 = ppsum_1.tile([1, 512], F32, tag="ln_ps")
            for do in range(DO):
                nc.tensor.matmul(out=ps2[:], lhsT=ones[:], rhs=sq[:, do, cs],
                                 start=(do == 0), stop=(do == DO - 1))
            # m2 = s2/D + eps
            nc.vector.tensor_scalar(out=m2[:, cs], in0=ps2[:], scalar1=1.0 / D,
                                    scalar2=EPS, op0=OP.mult, op1=OP.add)
        musq = psm.tile([1, S], F32, tag="ln_musq")
        nc.vector.tensor_tensor(out=musq[:], in0=mu[:], in1=mu[:], op=OP.mult)
        var = psm.tile([1, S], F32, tag="ln_var")
        nc.vector.tensor_tensor(out=var[:], in0=m2[:], in1=musq[:], op=OP.subtract)
        sd = psm.tile([1, S], F32, tag="ln_sd")
        nc.scalar.activation(out=sd[:], in_=var[:], func=AF.Sqrt)
        # rho (+ one NR polish: ACT Sqrt can be low-precision)
        rho = bc_src[:, 0:S]
        nc.vector.reciprocal(out=rho, in_=sd[:])
        pol = psm.tile([1, S], F32, tag="ln_pol")
        nc.vector.tensor_tensor(out=pol[:], in0=rho, in1=rho, op=OP.mult)
        nc.vector.tensor_tensor(out=pol[:], in0=pol[:], in1=var[:], op=OP.mult)
        nc.vector.tensor_scalar(out=pol[:], in0=pol[:], scalar1=-0.5, scalar2=1.5,
                                op0=OP.mult, op1=OP.add)
        nc.vector.tensor_tensor(out=rho, in0=rho, in1=pol[:], op=OP.mult)
        nc.vector.tensor_tensor(out=bc_src[:, S:2 * S], in0=mu[:], in1=rho,
                                op=OP.mult)
        bc = pw.tile([P, 2 * S], F32, tag="ln_bc")
        nc.gpsimd.partition_broadcast(bc[:], bc_src[:])

        tmp = pw.tile([P, DO, S], F32, tag="ln_scr")
        for do in range(DO):
            # normalize: split across GPSIMD (muls) and DVE (affine)
            nc.gpsimd.tensor_tensor(out=tmp[:, do, :], in0=src[:, do, :],
                                    in1=bc[:, 0:S], op=OP.mult)
            nc.gpsimd.tensor_tensor(out=tmp[:, do, :], in0=tmp[:, do, :],
                                    in1=bc[:, S:2 * S], op=OP.subtract)
            nc.vector.tensor_scalar(out=dst[:, do, :], in0=tmp[:, do, :],
                                    scalar1=g_sb[:, do:do + 1],
                                    scalar2=b_sb[:, do:do + 1],
                                    op0=OP.mult, op1=OP.add)


_DEBUG = False

_WPACK_BF = [("wqT", (P, DO, D)), ("wkT", (P, DO, D)), ("wvT", (P, DO, D)),
             ("woT", (P, DO, D)), ("w1T", (P, DO, F)), ("w2T", (P, FO, D))]
_WPACK_F32 = [("bq", (P, DO)), ("bk", (P, DO)), ("bv", (1, D)),
              ("bo", (P, DO)), ("bf1", (P, FO)), ("bf2", (P, DO)),
              ("g1", (P, DO)), ("b1", (P, DO)), ("g2", (P, DO)),
              ("b2", (P, DO)), ("gf", (P, DO)), ("bf", (P, DO))]


def _emit(nc, n_iters=1):
    """Emit the full encoder program. n_iters > 1 repeats the ENTIRE body
    (including every weight DMA) back-to-back on the same DRAM in/out
    buffers; used only by the timing harness to measure per-iteration HW
    time with the fixed per-execute dispatch overhead amortized away."""
    taps = {}

    def tap(name, ap_or_tile, shape, dt):
        if not _DEBUG:
            return
        d = nc.dram_tensor("tap_" + name, shape, dt, kind="ExternalOutput").ap()
        nc.sync.dma_start(d, ap_or_tile)
        taps[name] = d

    x_d = nc.dram_tensor("x", [S, D], F32, kind="ExternalInput").ap()
    # all weights packed into two flat buffers — the axon tunnel charges
    # ~90 ms per buffer round-trip, so fewer buffers >> anything else
    nbf = sum(int(np.prod(s)) for _, s in _WPACK_BF)
    nf = sum(int(np.prod(s)) for _, s in _WPACK_F32)
    wbf_d = nc.dram_tensor("wbf", [nbf], BF16, kind="ExternalInput").ap()
    wf_d = nc.dram_tensor("wf", [nf], F32, kind="ExternalInput").ap()

    def _slices(flat, pack):
        out, off = {}, 0
        for name, shape in pack:
            n = int(np.prod(shape))
            sl = flat[off:off + n]
            if len(shape) == 3:
                sl = sl.rearrange("(p o m) -> p o m", p=shape[0], o=shape[1])
            else:
                sl = sl.rearrange("(p o) -> p o", p=shape[0])
            out[name] = sl
            off += n
        return out

    dbf = _slices(wbf_d, _WPACK_BF)
    df = _slices(wf_d, _WPACK_F32)
    (wqT_d, wkT_d, wvT_d, woT_d, w1T_d, w2T_d) = (
        dbf["wqT"], dbf["wkT"], dbf["wvT"], dbf["woT"], dbf["w1T"], dbf["w2T"])
    (bq_d, bk_d, bv_d, bo_d, bf1_d, bf2_d, g1_d, b1_d, g2_d, b2_d, gf_d,
     bf_d) = (df["bq"], df["bk"], df["bv"], df["bo"], df["bf1"], df["bf2"],
              df["g1"], df["b1"], df["g2"], df["b2"], df["gf"], df["bf"])
    out_d = nc.dram_tensor("out", [S, D], F32, kind="ExternalOutput").ap()

    def _one_iter(tc):
      with ExitStack() as ctx:
        ppersist = ctx.enter_context(tc.tile_pool(name="persist", bufs=1))
        psmall = ctx.enter_context(tc.tile_pool(name="small", bufs=2))
        ppsum_z = ctx.enter_context(tc.tile_pool(name="psz", bufs=2, space="PSUM"))
        ppsum_g = ctx.enter_context(tc.tile_pool(name="psg", bufs=2, space="PSUM"))
        ppsum_1 = ctx.enter_context(tc.tile_pool(name="ps1", bufs=2, space="PSUM"))

        def load(pool, dram, shape, dt, tag):
            t = pool.tile(shape, dt, tag=tag)
            nc.sync.dma_start(t[:], dram)
            return t

        # ---- long-lived constants / tensors ----
        wo = load(ppersist, woT_d, [P, DO, D], BF16, "wo")
        bq = load(ppersist, bq_d, [P, DO], F32, "bq")
        bk = load(ppersist, bk_d, [P, DO], F32, "bk")
        bo = load(ppersist, bo_d, [P, DO], F32, "bo")
        bf1 = load(ppersist, bf1_d, [P, FO], F32, "bf1")
        bf2 = load(ppersist, bf2_d, [P, DO], F32, "bf2")
        g1 = load(ppersist, g1_d, [P, DO], F32, "g1")
        b1 = load(ppersist, b1_d, [P, DO], F32, "b1")
        g2 = load(ppersist, g2_d, [P, DO], F32, "g2")
        b2 = load(ppersist, b2_d, [P, DO], F32, "b2")
        gf = load(ppersist, gf_d, [P, DO], F32, "gf")
        bf = load(ppersist, bf_d, [P, DO], F32, "bf")
        bv1 = load(ppersist, bv_d, [1, D], F32, "bv1")
        bvbc = ppersist.tile([P, D], F32, tag="bvbc")
        nc.gpsimd.partition_broadcast(bvbc[:], bv1[:])

        ident = ppersist.tile([P, P], F32, tag="ident")
        make_identity(nc, ident[:])
        ones = ppersist.tile([P, 1], F32, tag="ones")
        nc.vector.memset(ones[:], 1.0)
        zer_bf = ppersist.tile([P, S], BF16, tag="zer_bf")
        nc.vector.memset(zer_bf[:], 0.0)
        zsc = ppersist.tile([P, 1], F32, tag="zsc")
        nc.vector.memset(zsc[:], 0.0)

        xT = ppersist.tile([P, DO, S], F32, tag="xT")          # reused as x3T
        x2T = ppersist.tile([P, DO, S], F32, tag="x2T")        # reused as outT
        attoutT = ppersist.tile([P, DO, S], BF16, tag="attoutT")

        with tc.tile_pool(name="attn_live", bufs=1) as pal:
            # ---- x load + transpose ----
            x_nat = pal.tile([P, TO, D], F32, tag="x_nat")
            nc.sync.dma_start(x_nat[:], x_d.rearrange("(to p) d -> p to d", p=P))
            for to in range(TO):
                for do in range(DO):
                    pt = ppsum_g.tile([P, 512], F32, tag="pg")
                    nc.tensor.transpose(pt[:, :P], x_nat[:, to, do * P:(do + 1) * P],
                                        ident[:])
                    nc.vector.tensor_copy(xT[:, do, to * P:(to + 1) * P], pt[:, :P])

            # ---- LN1 -> y (bf16, transposed) ----
            y_bf = pal.tile([P, DO, S], BF16, tag="y_bf")
            _ln_T(nc, tc, xT, y_bf, g1, b1, ones, ppsum_g, ppsum_1)
            tap("xT", xT[:], [P, DO, S], F32)
            tap("y_bf", y_bf[:], [P, DO, S], BF16)

            # ---- projections ----
            wq = load(pal, wqT_d, [P, DO, D], BF16, "wq")
            wk = load(pal, wkT_d, [P, DO, D], BF16, "wk")
            wv = load(pal, wvT_d, [P, DO, D], BF16, "wv")
            qT = pal.tile([P, DO, S], BF16, tag="qT")
            kT = pal.tile([P, DO, S], BF16, tag="kT")
            for (wmat, bias, dst) in ((wq, bq, qT), (wk, bk, kT)):
                for dt in range(DO):
                    for ch in range(2):
                        cs = slice(ch * 512, ch * 512 + 512)
                        ps = ppsum_g.tile([P, 512], F32, tag="pg")
                        for di in range(DO):
                            nc.tensor.matmul(
                                out=ps[:], lhsT=wmat[:, di, dt * P:(dt + 1) * P],
                                rhs=y_bf[:, di, cs],
                                start=(di == 0), stop=(di == DO - 1))
                        nc.vector.tensor_scalar(out=dst[:, dt, cs], in0=ps[:],
                                                scalar1=bias[:, dt:dt + 1],
                                                scalar2=None, op0=OP.add)
            v_bf = pal.tile([P, TO, D], BF16, tag="v_bf")
            for tt in range(TO):
                ps = ppsum_g.tile([P, 512], F32, tag="pg")
                for di in range(DO):
                    nc.tensor.matmul(out=ps[:], lhsT=y_bf[:, di, tt * P:(tt + 1) * P],
                                     rhs=wv[:, di, :],
                                     start=(di == 0), stop=(di == DO - 1))
                nc.vector.tensor_tensor(out=v_bf[:, tt, :], in0=ps[:], in1=bvbc[:],
                                        op=OP.add)
            tap("qT", qT[:], [P, DO, S], BF16)
            tap("kT", kT[:], [P, DO, S], BF16)
            tap("v_bf", v_bf[:], [P, TO, D], BF16)

            # ---- attention ----
            with tc.tile_pool(name="z", bufs=2) as pz, \
                 tc.tile_pool(name="attT", bufs=1) as pattT, \
                 tc.tile_pool(name="r", bufs=3) as pr, \
                 tc.tile_pool(name="r2", bufs=2) as pr2, \
                 tc.tile_pool(name="att", bufs=3) as patt:

                def emit_scores(h):
                    """q@kT for head h -> z bf16 [128, TO, S]; rows qt, cols kt."""
                    bp = (h % 2) * HD
                    doh = h // 2
                    q_l = qT[bp:bp + HD, doh, :]
                    k_r = kT[bp:bp + HD, doh, :]
                    z = pz.tile([P, TO, S], BF16, tag="z")
                    s1z = psmall.tile([P, TO], F32, tag="s1z")
                    s2z = psmall.tile([P, TO], F32, tag="s2z")
                    for qt in range(TO):
                        ps = ppsum_z.tile([P, S], F32, tag="pz")
                        for kc in range(2):
                            cs = slice(kc * 512, kc * 512 + 512)
                            nc.tensor.matmul(out=ps[:, cs],
                                             lhsT=q_l[:, qt * P:(qt + 1) * P],
                                             rhs=k_r[:, cs], start=True, stop=True)
                        if qt % 2 == 0 or qt == 1:
                            nc.scalar.activation(out=z[:, qt, :], in_=ps[:],
                                                 func=AF.Copy,
                                                 accum_out=s1z[:, qt:qt + 1])
                        else:
                            nc.vector.tensor_scalar(out=z[:, qt, :], in0=ps[:],
                                                    scalar1=1.0, scalar2=0.0,
                                                    op0=OP.mult, op1=OP.add,
                                                    accum_out=s1z[:, qt:qt + 1])
                        sub = pr.tile([P, S // 8], BF16, tag="sub")
                        nc.vector._custom_dve(
                            TENSOR_TENSOR_REDUCE, out=sub[:], in0=z[:, qt, ::8],
                            in1=z[:, qt, ::8], s0=0.0, s1=1.0,
                            accum_out=s2z[:, qt:qt + 1])
                    return z, s1z, s2z

                def emit_entmax(h, z, s1z, s2z):
                    # --- init: tau0 = m - u(w)*sigma,  w = sqrt(1/(S*var)) ---
                    m = psmall.tile([P, TO], F32, tag="tm")
                    nc.vector.tensor_scalar(out=m[:], in0=s1z[:], scalar1=1.0 / S,
                                            scalar2=None, op0=OP.mult)
                    msq = psmall.tile([P, TO], F32, tag="tmsq")
                    nc.vector.tensor_tensor(out=msq[:], in0=m[:], in1=m[:], op=OP.mult)
                    var = psmall.tile([P, TO], F32, tag="tvar")
                    nc.vector.tensor_scalar(out=var[:], in0=s2z[:], scalar1=8.0 / S,
                                            scalar2=None, op0=OP.mult)
                    nc.vector.tensor_tensor(out=var[:], in0=var[:], in1=msq[:],
                                            op=OP.subtract)
                    nc.vector.tensor_scalar(out=var[:], in0=var[:], scalar1=1e-8,
                                            scalar2=None, op0=OP.max)
                    th = psmall.tile([P, TO], F32, tag="tth")
                    nc.vector.reciprocal(out=th[:], in_=var[:])
                    nc.vector.tensor_scalar(out=th[:], in0=th[:], scalar1=1.0 / S,
                                            scalar2=None, op0=OP.mult)
                    w = psmall.tile([P, TO], F32, tag="tw")
                    nc.scalar.activation(out=w[:], in_=th[:], func=AF.Sqrt)
                    nc.vector.tensor_scalar(out=w[:], in0=w[:], scalar1=W_LO,
                                            scalar2=W_HI, op0=OP.max, op1=OP.min)
                    sg = psmall.tile([P, TO], F32, tag="tsg")
                    nc.scalar.activation(out=sg[:], in_=var[:], func=AF.Sqrt)
                    u = psmall.tile([P, TO], F32, tag="tu")
                    nc.vector.tensor_scalar(out=u[:], in0=w[:], scalar1=UC3,
                                            scalar2=UC2, op0=OP.mult, op1=OP.add)
                    nc.vector.tensor_tensor(out=u[:], in0=u[:], in1=w[:], op=OP.mult)
                    nc.vector.tensor_scalar(out=u[:], in0=u[:], scalar1=UC1,
                                            scalar2=None, op0=OP.add)
                    nc.vector.tensor_tensor(out=u[:], in0=u[:], in1=w[:], op=OP.mult)
                    nc.vector.tensor_scalar(out=u[:], in0=u[:], scalar1=UC0,
                                            scalar2=None, op0=OP.add)
                    # keep tau NEGATED: tau_neg = u*sigma - m, so both DVE
                    # (STT op0=add) and ACT (bias=tau_neg) can consume it
                    tau = psmall.tile([P, TO], F32, tag="tau")
                    nc.vector.tensor_tensor(out=tau[:], in0=u[:], in1=sg[:],
                                            op=OP.mult)
                    nc.vector.tensor_tensor(out=tau[:], in0=tau[:], in1=m[:],
                                            op=OP.subtract)

                    # --- Newton iterations ---
                    # Engine split per tile: qt<4 -> ACT Relu(+S1) then DVE
                    # custom-TTR square(+S2); qt>=4 -> DVE STT relu(+S1) then
                    # ACT Square(+S2). 8 big ops per engine per head-iter.
                    attT = pattT.tile([P, TO, S], BF16, tag="attT")
                    for it in range(NEWTON_ITERS):
                        last = FUSE_FINAL and (it == NEWTON_ITERS - 1)
                        # progressive sampling: quarter, half, full keys --
                        # early Newton steps tolerate sampled sums (validated
                        # vs sort-based entmax: rms unchanged)
                        ncols = S if last else (S // 4 if it == 0 else S // 2)
                        s1 = psmall.tile([P, TO], F32, tag="ns1")
                        s2 = psmall.tile([P, TO], F32, tag="ns2")
                        for qt in range(TO):
                            r = pr.tile([P, S], BF16, tag="r")
                            rv = r[:, 0:ncols]
                            zv = z[:, qt, 0:ncols]
                            if last and qt >= 2:
                                # no S1 needed in the final pass: 1-stream
                                # AP+AP tensor_scalar relu (HW-probed)
                                nc.vector.tensor_scalar(
                                    out=rv, in0=zv,
                                    scalar1=tau[:, qt:qt + 1], scalar2=zsc[:],
                                    op0=OP.add, op1=OP.max)
                            elif qt < 4:
                                # r = relu(z + tau_neg) on ACT, S1 accumulated
                                nc.scalar.activation(
                                    out=rv, in_=zv, func=AF.Relu,
                                    bias=tau[:, qt:qt + 1], scale=1.0,
                                    accum_out=None if last else s1[:, qt:qt + 1])
                            else:
                                nc.vector.scalar_tensor_tensor(
                                    out=rv, in0=zv,
                                    scalar=tau[:, qt:qt + 1],
                                    in1=zer_bf[:, 0:ncols],
                                    op0=OP.add, op1=OP.max,
                                    accum_out=s1[:, qt:qt + 1])
                            if last:
                                r2 = patt.tile([P, S], BF16, tag="arow")
                            elif qt < 4:
                                r2 = patt.tile([P, S], BF16, tag="arow")
                            else:
                                r2 = pr2.tile([P, S], F32, tag="r2f")
                            r2v = r2[:, 0:ncols]
                            if last and qt >= 6:
                                # final squares: Pool SBUF multiply (legal)
                                nc.gpsimd.tensor_tensor(
                                    out=r2v, in0=rv, in1=rv, op=OP.mult)
                            elif qt < 4:
                                nc.vector._custom_dve(
                                    TENSOR_TENSOR_REDUCE, out=r2v, in0=rv,
                                    in1=rv, s0=0.0, s1=1.0,
                                    accum_out=s2[:, qt:qt + 1])
                            else:
                                nc.scalar.activation(out=r2v, in_=rv,
                                                     func=AF.Square,
                                                     accum_out=s2[:, qt:qt + 1])
                            if last:
                                nc.sync.dma_start_transpose(
                                    attT[:, :, qt * P:(qt + 1) * P], r2[:])
                        if last:
                            break
                        # tau_neg -= clip((s2-c)/(2*s1), 0, 0.25); the it==0
                        # pass sums only the first half of the keys, so its
                        # sums estimate half the full values: c = 0.5.
                        cnum = -0.125 if it == 0 else -0.25
                        rcp = psmall.tile([P, TO], F32, tag="nrcp")
                        nc.vector.reciprocal(out=rcp[:], in_=s1[:])
                        num = psmall.tile([P, TO], F32, tag="nnum")
                        nc.vector.tensor_scalar(out=num[:], in0=s2[:], scalar1=0.5,
                                                scalar2=cnum, op0=OP.mult, op1=OP.add)
                        step = psmall.tile([P, TO], F32, tag="nstep")
                        nc.vector.tensor_tensor(out=step[:], in0=num[:], in1=rcp[:],
                                                op=OP.mult)
                        nc.vector.tensor_scalar(out=step[:], in0=step[:], scalar1=0.0,
                                                scalar2=0.25, op0=OP.max, op1=OP.min)
                        nc.vector.tensor_tensor(out=tau[:], in0=tau[:], in1=step[:],
                                                op=OP.subtract)
                    return attT

                def emit_attv(h, attT):
                    bp = (h % 2) * HD
                    doh = h // 2
                    for ch in range(2):
                        cs = slice(ch * 512, ch * 512 + 512)
                        ps = ppsum_g.tile([P, 512], F32, tag="pg")
                        for kto in range(TO):
                            nc.tensor.matmul(out=ps[:HD, :],
                                             lhsT=v_bf[:, kto, h * HD:(h + 1) * HD],
                                             rhs=attT[:, kto, cs],
                                             start=(kto == 0), stop=(kto == TO - 1))
                        nc.vector.tensor_copy(attoutT[bp:bp + HD, doh, cs], ps[:HD, :])

                pending = (0,) + emit_scores(0)
                for h in range(H):
                    _, z, s1z, s2z = pending
                    if h == 0:
                        tap("z0", z[:], [P, TO, S], BF16)
                        tap("s1z0", s1z[:], [P, TO], F32)
                        tap("s2z0", s2z[:], [P, TO], F32)
                    attT = emit_entmax(h, z, s1z, s2z)
                    if h == 0:
                        tap("attT0", attT[:], [P, TO, S], BF16)
                    if h + 1 < H:
                        pending = (h + 1,) + emit_scores(h + 1)
                    emit_attv(h, attT)
                tap("attoutT", attoutT[:], [P, DO, S], BF16)

        # ---- output projection + residual: x2T = xT + woT.T @ attoutT + bo ----
        for dt in range(DO):
            for ch in range(2):
                cs = slice(ch * 512, ch * 512 + 512)
                ps = ppsum_g.tile([P, 512], F32, tag="pg")
                for di in range(DO):
                    nc.tensor.matmul(out=ps[:], lhsT=wo[:, di, dt * P:(dt + 1) * P],
                                     rhs=attoutT[:, di, cs],
                                     start=(di == 0), stop=(di == DO - 1))
                nc.vector.scalar_tensor_tensor(
                    out=x2T[:, dt, cs], in0=ps[:], scalar=bo[:, dt:dt + 1],
                    in1=xT[:, dt, cs], op0=OP.add, op1=OP.add)

        with tc.tile_pool(name="ffn_live", bufs=1) as pfl:
            w1 = load(pfl, w1T_d, [P, DO, F], BF16, "w1")
            w2 = load(pfl, w2T_d, [P, FO, D], BF16, "w2")
            y2_bf = pfl.tile([P, DO, S], BF16, tag="y2_bf")
            hT = pfl.tile([P, FO, S], BF16, tag="hT")

            tap("x2T", x2T[:], [P, DO, S], F32)
            # ---- LN2 -> y2 ----
            _ln_T(nc, tc, x2T, y2_bf, g2, b2, ones, ppsum_g, ppsum_1)

            # ---- FFN in: hT = mish(w1T.T @ y2 + bf1) ----
            with tc.tile_pool(name="mish", bufs=2) as pm:
                for fo in range(FO):
                    for ch in range(2):
                        cs = slice(ch * 512, ch * 512 + 512)
                        ps = ppsum_g.tile([P, 512], F32, tag="pg")
                        for di in range(DO):
                            nc.tensor.matmul(
                                out=ps[:], lhsT=w1[:, di, fo * P:(fo + 1) * P],
                                rhs=y2_bf[:, di, cs],
                                start=(di == 0), stop=(di == DO - 1))
                        e = pm.tile([P, 512], F32, tag="m_e")
                        nc.scalar.activation(out=e[:], in_=ps[:], func=AF.Exp,
                                             bias=bf1[:, fo:fo + 1], scale=1.0)
                        xb = pm.tile([P, 512], F32, tag="m_xb")
                        nc.scalar.activation(out=xb[:], in_=ps[:], func=AF.Identity,
                                             bias=bf1[:, fo:fo + 1], scale=1.0)
                        a = pm.tile([P, 512], F32, tag="m_a")
                        nc.vector.scalar_tensor_tensor(out=a[:], in0=e[:], scalar=2.0,
                                                       in1=e[:], op0=OP.add,
                                                       op1=OP.mult)
                        d = pm.tile([P, 512], F32, tag="m_d")
                        nc.vector.tensor_scalar(out=d[:], in0=a[:], scalar1=2.0,
                                                scalar2=None, op0=OP.add)
                        rc = pm.tile([P, 512], F32, tag="m_rc")
                        nc.vector.reciprocal_approx_fast(out=rc[:], in_=d[:])
                        p1 = pm.tile([P, 512], F32, tag="m_p1")
                        nc.gpsimd.tensor_tensor(out=p1[:], in0=xb[:], in1=a[:],
                                                op=OP.mult)
                        nc.gpsimd.tensor_tensor(out=hT[:, fo, cs], in0=p1[:],
                                                in1=rc[:], op=OP.mult)

            # ---- FFN out + residual: x3T = x2T + w2T.T @ hT + bf2 ----
            x3T = ppersist.tile([P, DO, S], F32, tag="xT")  # reuse xT slot
            for dt in range(DO):
                for ch in range(2):
                    cs = slice(ch * 512, ch * 512 + 512)
                    ps = ppsum_g.tile([P, 512], F32, tag="pg")
                    for fo in range(FO):
                        nc.tensor.matmul(out=ps[:],
                                         lhsT=w2[:, fo, dt * P:(dt + 1) * P],
                                         rhs=hT[:, fo, cs],
                                         start=(fo == 0), stop=(fo == FO - 1))
                    nc.vector.scalar_tensor_tensor(
                        out=x3T[:, dt, cs], in0=ps[:], scalar=bf2[:, dt:dt + 1],
                        in1=x2T[:, dt, cs], op0=OP.add, op1=OP.add)

            tap("hT", hT[:], [P, FO, S], BF16)
            tap("x3T", x3T[:], [P, DO, S], F32)

            # ---- final LN (outT reuses the x2T slot) ----
            outT = ppersist.tile([P, DO, S], F32, tag="x2T")
            _ln_T(nc, tc, x3T, outT, gf, bf, ones, ppsum_g, ppsum_1)

            # ---- transpose back + store ----
            with tc.tile_pool(name="outp", bufs=1) as po:
                out_nat = po.tile([P, TO, D], F32, tag="out_nat")
                for to in range(TO):
                    for do in range(DO):
                        pt = ppsum_g.tile([P, 512], F32, tag="pg")
                        nc.tensor.transpose(pt[:, :P],
                                            outT[:, do, to * P:(to + 1) * P],
                                            ident[:])
                        nc.vector.tensor_copy(out_nat[:, to, do * P:(do + 1) * P],
                                              pt[:, :P])
                nc.sync.dma_start(out_d.rearrange("(to p) d -> p to d", p=P),
                                  out_nat[:])

    with tile.TileContext(nc) as tc:
        for _ in range(n_iters):
            _one_iter(tc)

    return nc


_CACHE = {}


def _get_nc(n_iters=1):
    key = "nc" if n_iters == 1 else f"nc{n_iters}"
    if key not in _CACHE:
        nc = bacc.Bacc("TRN2", target_bir_lowering=False, debug=False)
        _emit(nc, n_iters=n_iters)
        nc.compile()
        _CACHE[key] = nc
    return _CACHE[key]


def _prep_weights(inputs):
    bf = ml_dtypes.bfloat16
    c = 1.0 / 16.0  # 1/(2*sqrt(hd)) folded into q

    def tr(w):  # [dout, din] -> [din(P,O), dout]
        wt = np.ascontiguousarray(np.asarray(w, dtype=np.float32).T)
        o = wt.shape[0] // P
        return np.ascontiguousarray(wt.reshape(o, P, -1).transpose(1, 0, 2))

    def col(v):  # [n] -> [P, n//P] per-partition layout
        return np.ascontiguousarray(
            np.asarray(v, dtype=np.float32).reshape(-1, P).T)

    vals = {
        "wqT": tr(np.asarray(inputs["Wq"]) * c).astype(bf),
        "wkT": tr(inputs["Wk"]).astype(bf),
        "wvT": tr(inputs["Wv"]).astype(bf),
        "woT": tr(inputs["Wo"]).astype(bf),
        "w1T": tr(inputs["W1"]).astype(bf),
        "w2T": tr(inputs["W2"]).astype(bf),
        "bq": col(np.asarray(inputs["bq"]) * c),
        "bk": col(inputs["bk"]),
        "bv": np.asarray(inputs["bv"], dtype=np.float32).reshape(1, -1).copy(),
        "bo": col(inputs["bo"]),
        "bf1": col(inputs["bf1"]),
        "bf2": col(inputs["bf2"]),
        "g1": col(inputs["ln1_g"]),
        "b1": col(inputs["ln1_b"]),
        "g2": col(inputs["ln2_g"]),
        "b2": col(inputs["ln2_b"]),
        "gf": col(inputs["lnf_g"]),
        "bf": col(inputs["lnf_b"]),
    }
    wbf = np.concatenate([np.ascontiguousarray(vals[n]).ravel()
                          for n, _ in _WPACK_BF])
    wf = np.concatenate([np.ascontiguousarray(vals[n]).ravel()
                         for n, _ in _WPACK_F32])
    return {"wbf": wbf, "wf": wf}


def _get_runner(n_cores, n_iters=1):
    """Build the shard_map'd jit callable once and reuse it across calls
    (run_bass_via_pjrt re-traces per call, which costs ~100ms)."""
    key = ("runner", n_cores, n_iters)
    if key in _CACHE:
        return _CACHE[key]
    import jax
    import numpy as _np
    from jax.sharding import Mesh, PartitionSpec
    from jax.experimental.shard_map import shard_map
    from concourse import bass2jax as b2j
    from concourse import mybir as mb

    nc = _get_nc(n_iters)
    b2j.install_neuronx_cc_hook()
    pid_name = nc.partition_id_tensor.name if nc.partition_id_tensor else None
    in_names, out_names, out_avals, zero_shapes = [], [], [], []
    for alloc in nc.m.functions[0].allocations:
        if not isinstance(alloc, mb.MemoryLocationSet):
            continue
        name = alloc.memorylocations[0].name
        if alloc.kind == "ExternalInput":
            if name != pid_name:
                in_names.append(name)
        elif alloc.kind == "ExternalOutput":
            out_names.append(name)
            shape = tuple(alloc.tensor_shape)
            dtype = mb.dt.np(alloc.dtype)
            out_avals.append(jax.core.ShapedArray(shape, dtype))
            zero_shapes.append((shape, dtype))
    n_params = len(in_names)
    all_names = in_names + out_names
    if pid_name is not None:
        all_names = all_names + [pid_name]
    donate = tuple(range(n_params, n_params + len(out_names)))

    def _body(*args):
        operands = list(args)
        if pid_name is not None:
            operands.append(b2j.partition_id_tensor())
        outs = b2j._bass_exec_p.bind(
            *operands,
            out_avals=tuple(out_avals),
            in_names=tuple(all_names),
            out_names=tuple(out_names),
            lowering_input_output_aliases=(),
            sim_require_finite=True,
            sim_require_nnan=True,
            nc=nc,
        )
        return tuple(outs)

    devices = jax.devices()[:n_cores]
    mesh = Mesh(_np.asarray(devices), ("core",))
    # only "x" differs per core; every weight/bias is replicated so the
    # host->device upload ships one copy instead of n_cores concatenated ones
    sharded_names = {"x"}
    in_specs = tuple(
        PartitionSpec("core") if n in sharded_names else PartitionSpec()
        for n in in_names
    ) + (PartitionSpec("core"),) * len(out_names)
    sharded = jax.jit(
        shard_map(_body, mesh=mesh, in_specs=in_specs,
                  out_specs=(PartitionSpec("core"),) * len(out_names),
                  check_rep=False),
        donate_argnums=donate, keep_unused=True)

    # donated output buffers are created ON DEVICE (the kernel writes every
    # output element, so their content is irrelevant; uploading 16 MB of
    # host zeros per call would cost ~200 ms through the axon tunnel)
    from jax.sharding import NamedSharding
    import jax.numpy as jnp
    zshard = NamedSharding(mesh, PartitionSpec("core"))
    zeros_maker = jax.jit(
        lambda: tuple(jnp.zeros((n_cores * s[0],) + tuple(s[1:]), dt)
                      for (s, dt) in zero_shapes),
        out_shardings=(zshard,) * len(zero_shapes))

    runner = (sharded, in_names, out_names, zero_shapes, n_cores, sharded_names,
              zeros_maker)
    _CACHE[key] = runner
    return runner


def _run(in_maps):
    import numpy as _np
    (sharded, in_names, out_names, zero_shapes, n_cores, sharded_names,
     zeros_maker) = _get_runner(len(in_maps))
    concat_in = [
        _np.concatenate([_np.asarray(m[name]) for m in in_maps], axis=0)
        if name in sharded_names else _np.asarray(in_maps[0][name])
        for name in in_names
    ]
    zeros = zeros_maker()
    outs = sharded(*concat_in, *zeros)
    res = []
    for c in range(n_cores):
        d = {}
        for i, name in enumerate(out_names):
            arr = _np.asarray(outs[i])
            per = arr.shape[0] // n_cores
            d[name] = arr[c * per:(c + 1) * per]
        res.append(d)
    return res


def kernel(**inputs) -> np.ndarray:
    x = np.asarray(inputs["x"], dtype=np.float32)
    B = x.shape[0]
    shared = _prep_weights(inputs)
    in_maps = []
    for b in range(B):
        m = dict(shared)
        m["x"] = np.ascontiguousarray(x[b])
        in_maps.append(m)
    results = _run(in_maps)
    out = np.stack([results[b]["out"] for b in range(B)], axis=0)
    return out.astype(np.float32)


if __name__ == "__main__":
    import reference
    inputs = reference.setup_inputs()
    outs = kernel(**{k: np.asarray(v) for k, v in inputs.items()})
    print("kernel output:", outs.shape, outs.dtype)

